# revision 54
# baseline (speedup 1.0000x reference)
"""Chamfer loss kernel for Trainium2 (8 NeuronCores, data-parallel over batch).

Problem: B=8, N=M=4096, D=3 fp32 point clouds.
  loss = mean_b mean_n min_m ||p_bn - g_bm||^2  +  mean_b mean_m min_n ||.||^2
  (squared euclidean, clamped at 0, matching pytorch3d norm=2 semantics)

Strategy (one batch element per core):
  - Distance tiles d[n, m] come from K=5 float32r matmuls on the PE:
       k0: 1 * |g|^2_hi   k1: 1 * |g|^2_lo   k2-4: (-2 p_d) * g_d
    The pred-point squared norm |p_n|^2 is constant per OUTPUT ROW
    (= PSUM partition), so it rides the ACT relu-copy's per-partition
    bias port in exact fp32 instead of costing two more f32r strip rows
    (and the whole pred-side norm hi/lo+transpose prep chain). f32r keeps
    ~12 mantissa bits; the gt norms are hi/lo split so d's error is only
    the coordinate-rounding perturbation (~1e-5 relative on the loss).
  - The K=5 operand strips are replicated at the 4 PE row-group bases
    (partition 32g) via 6 SBUF-SBUF DMAs on the SP/GPSIMD queues; each
    half-span's 4 matmuls then come from 4 different row groups, so the
    PE reorder window hides LDWEIGHTS (PE busy ~131us unpacked -> ~70us).
    Tiles t<6 run unpacked from row group 0 so nothing waits on the
    replication; doorbells live only on the SP/GPSIMD sequencers because
    a doorbell's dependency wait stalls its whole in-order sequencer.
  - ACT relu-copies each PSUM half-span to SBUF bf16 (clamp + |p|^2 bias
    fused; ~3.7us/tile, hidden under the DVE).
  - Row minima: deep bf16 tensor_tensor fold-tree (2x mode) over groups
    of 2-4 tiles down to width 64, then one 1x tensor_reduce
    (~2.2-2.3us/tile). Tiles 0/1 fold per half-span so the DVE starts on
    the very first RELU.
  - Column minima: [128, 2, M] tensor_tensor min into two interleaved
    accumulators (~2.2us/tile) written directly by tiles 0/1's RELUs (no
    init). They merge while tiles 30-31 stream; those two tiles update
    the merged accumulator as singles (tile 31 in halves) so the
    end-chain after the last RELU is short. Partition-axis min: 2 groups
    of 16 PE transposes; ACT lifts each group to SBUF so the reduce runs
    as a 2x bf16 fold + short reduce. Row+col sums fuse into one reduce.
  - The DVE is the bottleneck engine (~151us busy, ~99% dense from the
    first fold to the end); ACT ~122us; PE ~70us. Engine-assignment notes
    for this toolchain: tensor_tensor/tensor_reduce(free-axis) are
    DVE-only (Pool rejects them in walrus codegen), native
    TENSOR_TENSOR_REDUCE min/min fails ISA encoding, DMA CCE accum
    supports no min, InstPool max measures 1.6 cyc/elem (slower than the
    fold tree), matmul bf16 PSUM output is TRN3-only, and ACT's
    accumulator is sum-only - so the bf16 fold-tree/col-acc floor
    (~4.4us/tile, at the DVE write-port limit) is the steady-state wall.
  - Per-core scalar output (cham_x_b + cham_y_b); the host averages the 8
    per-core scalars (the data-parallel gather).

All arithmetic happens on-chip; the host only reshapes/transposes inputs
(layout) and averages the per-core partial losses (unshard).
"""

import os
import sys

import numpy as np

sys.path.insert(0, "/opt/trn_rl_repo")

import bass_rust
import concourse.bass as bass
import concourse.mybir as mybir
from concourse.bass_utils import run_bass_kernel_spmd
from concourse.masks import make_identity
from concourse.tile import TileContext

B, N, M, D = 8, 4096, 4096, 3
NT = N // 128  # 32 n-tiles
K = 5
F32 = mybir.dt.float32
F32R = mybir.dt.float32r
BF16 = mybir.dt.bfloat16
BIG = 3.0e38

# ---------------------------------------------------------------------------
# walrus in this container rejects >1 sync-wait per instruction; spill the
# extras onto engine-matched NoOps placed immediately before the instruction.
_nop_counter = [0]


def _split_multi_waits(nc):
    for func in nc.m.functions:
        for bb in func.blocks:
            out = []
            dirty = False
            for inst in bb.instructions:
                si = inst.sync_info
                if si is not None and len(si.on_wait) > 1:
                    waits = list(si.on_wait)
                    for w in waits[:-1]:
                        _nop_counter[0] += 1
                        nop = mybir.InstNoOp(
                            name=f"I-waitsplit-{_nop_counter[0]}", ins=[], outs=[]
                        )
                        nop.engine = inst.engine
                        nop.sync_info = bass_rust.SyncInfo(on_wait=[w], on_update=[])
                        out.append(nop)
                    inst.sync_info = bass_rust.SyncInfo(
                        on_wait=[waits[-1]], on_update=list(si.on_update)
                    )
                    dirty = True
                out.append(inst)
            if dirty:
                bb.instructions = out
    return nc


# ---------------------------------------------------------------------------


_PREP_WR = {}


def _build_prep_side(nc, tc, pool, zpk, w_dram, ident, scale, c_row, q, psp, strip_norms):
    """Build one side's coord rows (and optionally norm rows) of zpk
    [96+K, 4096] f32r.

    Everything derives from the wide input layout [128, 96] (point 128t+p at
    partition p, cols 3t..3t+2): rounded coords via PE-transpose flattened
    into n-order by ONE 3-row reshape DMA. For the gt side (strip_norms),
    the squared norms are hi/lo-split into f32r strip rows 0-1; for the pred
    side the wide fp32 norms tile is returned instead and applied later as
    the per-partition bias of the ACT relu-copy (exact fp32, no strip rows).
    Row groups 1-3 are filled by the replication hops in build_nc.
    """
    nm = w_dram.name
    wr = _PREP_WR[nm]  # rounded+scaled wide input, loaded up front

    # --- coords: transpose wr -> [96, 128], one 3-row reshape DMA ---
    tw_ps = psp.tile([128, 128], F32, name=f"twps_{nm}", tag="ps_main")
    nc.tensor.matmul(
        tw_ps[0 : 3 * NT, :],
        wr.bitcast(F32),
        ident,
        is_transpose=True,
        start=True,
        stop=True,
    )
    tw = pool.tile([3 * NT, 128], F32R, name=f"tw_{nm}")
    nc.vector.tensor_copy(tw, tw_ps[0 : 3 * NT, :])
    tw_d = tw.rearrange("(t d) p -> d t p", d=3)
    for d in range(3):
        q.dma_start(out=zpk[c_row + d : c_row + d + 1, :], in_=tw_d[d])

    # --- norms of the rounded points ---
    wsq = pool.tile([128, 3 * NT], F32, name=f"wsq_{nm}")
    nc.vector.tensor_mul(wsq, wr, wr)
    norms = pool.tile([128, NT], F32, name=f"norms_{nm}")
    nc.vector.tensor_reduce(
        out=norms,
        in_=wsq.rearrange("p (t d) -> p t d", d=3),
        axis=mybir.AxisListType.X,
        op=mybir.AluOpType.add,
    )
    if scale != 1.0:
        # norms of scale*p -> divide by scale^2 (exact for powers of 2)
        nc.vector.tensor_scalar(
            out=norms,
            in0=norms,
            scalar1=1.0 / (scale * scale),
            scalar2=None,
            op0=mybir.AluOpType.mult,
        )
    if not strip_norms:
        return norms
    nh = pool.tile([128, NT], F32R, name=f"nh_{nm}")
    nc.vector.tensor_copy(nh, norms)
    nl_f = pool.tile([128, NT], F32, name=f"nlf_{nm}")
    nc.vector.tensor_sub(nl_f, norms, nh.bitcast(F32))
    nl = pool.tile([128, NT], F32R, name=f"nl_{nm}")
    nc.vector.tensor_copy(nl, nl_f)
    tn2 = pool.tile([2 * NT, 128], F32R, name=f"tn2_{nm}")
    for i, src in enumerate((nh, nl)):
        tn_ps = psp.tile([128, 128], F32, name=f"tnps_{nm}_{i}", tag="ps_main")
        nc.tensor.matmul(
            tn_ps[0:NT, :],
            src.bitcast(F32),
            ident,
            is_transpose=True,
            start=True,
            stop=True,
        )
        nc.vector.tensor_copy(tn2[NT * i : NT * (i + 1), :], tn_ps[0:NT, :])
    q.dma_start(out=zpk[0:2, :], in_=tn2)
    return None


def build_nc():
    nc = bass.Bass("TRN2")
    predW = nc.dram_tensor("predW", [128, 3 * NT], F32, kind="ExternalInput")
    gtW = nc.dram_tensor("gtW", [128, 3 * NT], F32, kind="ExternalInput")
    out_d = nc.dram_tensor("out", [1, 1], F32, kind="ExternalOutput")

    with TileContext(nc) as tc:
        with (
            tc.tile_pool(name="persist", bufs=1) as persist,
            tc.tile_pool(name="dsb", bufs=2) as dsbp,
        ):
            # load + round both wide inputs first — everything derives from
            # them, so they must not queue behind prep DMAs
            for w_dram, scale, q in ((predW, -2.0, nc.sync), (gtW, 1.0, nc.scalar)):
                w_in = persist.tile([128, 3 * NT], F32, name=f"w_{w_dram.name}")
                q.dma_start(out=w_in, in_=w_dram.ap())
                wr_t = persist.tile([128, 3 * NT], F32R, name=f"wr_{w_dram.name}")
                nc.vector.tensor_scalar(
                    out=wr_t,
                    in0=w_in,
                    scalar1=scale,
                    scalar2=None,
                    op0=mybir.AluOpType.mult,
                )
                _PREP_WR[w_dram.name] = wr_t
            # identity zero-fill on DVE so GPSIMD's single queue only does
            # the diagonal writes (keeps the prep critical path short)
            ident = persist.tile([128, 128], F32)
            nc.vector.memset(ident, 0.0)
            make_identity(nc, ident, nomemset=True)
            ident16 = persist.tile([128, 128], BF16)
            nc.vector.memset(ident16, 0.0)
            make_identity(nc, ident16, nomemset=True)

            # K=5 operand strips, replicated at the 4 PE row-group bases so
            # consecutive matmuls hide their LDWEIGHTS across row groups.
            zp_pk = persist.tile([96 + K, N], F32R)
            zg_pk = persist.tile([96 + K, M], F32R)
            # ones rows (Memset can't target f32r; compute ops can't start at
            # unaligned partitions): rounded ones strip -> broadcast DMA.
            # ones rows, built wide (128 lanes, ~100ns) then DMA'd out
            ones_w = persist.tile([128, 64], F32)
            nc.vector.memset(ones_w, 1.0)
            ones_wr = persist.tile([128, 64], F32R)
            nc.vector.tensor_copy(ones_wr, ones_w)
            nc.sync.dma_start(out=zp_pk[0:2, :], in_=ones_wr)
            # zp rows: [1, 1, -2p0, -2p1, -2p2]   (pnorm applied as ACT bias)
            # zg rows: [nGh, nGl, g0, g1, g2]
            psp = tc.alloc_tile_pool(name="psum_all", bufs=2, space="PSUM")
            _build_prep_side(
                nc, tc, persist, zg_pk, gtW, ident, 1.0, 2,
                nc.scalar, psp, strip_norms=True,
            )
            pnorms = _build_prep_side(
                nc, tc, persist, zp_pk, predW, ident, -2.0, 2,
                nc.sync, psp, strip_norms=False,
            )
            # replicate both K-strips at PE row-group bases 32/64/96: the 4
            # matmuls of each half-span then come from 4 different row groups,
            # so the PE reorder window hides each group's LDWEIGHTS under the
            # others' streams (PE busy ~131us unpacked -> ~59us packed)
            # doorbells only on SP/GPSIMD queues: a doorbell's dependency wait
            # blocks its whole (in-order) sequencer, and Scalar must keep
            # issuing the main-loop ACT copies
            repl_q = [nc.sync, nc.gpsimd, nc.gpsimd, nc.sync, nc.sync, nc.gpsimd]
            for g in range(1, 4):
                repl_q[2 * (g - 1)].dma_start(
                    out=zp_pk[32 * g : 32 * g + K, :], in_=zp_pk[0:K, :]
                )
                repl_q[2 * (g - 1) + 1].dma_start(
                    out=zg_pk[32 * g : 32 * g + K, :], in_=zg_pk[0:K, :]
                )

            rowcol = persist.tile([128, 2 * NT], F32)
            rowmins = rowcol[:, 0:NT]
            # half-tile row-min slots for tiles 0/1 (folded per half-span so
            # the DVE starts as soon as the very first RELU lands)
            rowminsH = persist.tile([128, 4], F32)
            # two independent column accumulators (updated as one [128,2,M]
            # tensor_tensor, halving the op count) merged once at the end
            colacc2 = persist.tile([128, 2 * M], BF16)
            colacc = persist.tile([128, M], BF16)
            colacc2_v = colacc2.rearrange("p (i m) -> p i m", m=M)
            # fold-tree intermediates: written+read only by the DVE (in-order)
            # so single buffers suffice; sized for the max group of 4 tiles
            foldp = tc.alloc_tile_pool(name="foldp", bufs=1)
            ftiles = [
                foldp.tile([128, 4, w], BF16, name=f"f{w}")
                for w in (2048, 1024, 512, 256, 128, 64)
            ]

            # ---- main loop: 32 n-tiles x 2 half-spans of [128, 2048];
            #      each span = 4 row-group-packed concurrent matmuls ----
            # Pipeline per n-tile: PE (4 packed MMs per half-span) -> ACT
            # relu-copies PSUM->SBUF bf16 (clamp fused, so no tail clamps)
            # -> DVE row path: deep bf16 fold-tree (2x mode) over groups of
            # 4 tiles down to width 64 before the (1x-rate) tensor_reduce;
            # col path: [128,2,M] tensor_tensor min into the paired
            # accumulators. Steady-state DVE ~4.4us/tile vs ~4.7 baseline.
            def emit_spans(t, dsb, base, dve_h1=False):
                # early tiles run unpacked from row group 0 so they don't
                # wait on the replication DMAs; later tiles rotate across
                # the 4 row groups (LDWEIGHTS hidden, PE ~60us busy)
                packed = t >= 6
                for h in range(2):
                    ps = psp.tile([128, 2048], F32, name="ps_main", tag="ps_main")
                    for g in range(4):
                        col0 = 2048 * h + 512 * g
                        b = 32 * g if packed else 0
                        nc.tensor.matmul(
                            ps[:, 512 * g : 512 * (g + 1)],
                            zp_pk[b : b + K, 128 * t : 128 * (t + 1)],
                            zg_pk[b : b + K, col0 : col0 + 512],
                            start=True,
                            stop=True,
                            tile_position=(b, 0),
                        )
                    dst = dsb[:, base + 2048 * h : base + 2048 * (h + 1)]
                    # the pred-point squared norm is constant per output row
                    # (= partition), so it rides the ACT copy's bias port in
                    # exact fp32 instead of costing two f32r strip rows
                    if dve_h1 and h == 1:
                        # warmup assist: the DVE converts this half during a
                        # window where it would otherwise idle waiting on ACT
                        # deliveries, pulling the whole schedule forward. No
                        # relu here - the final rowcol clamp covers it.
                        nc.vector.tensor_scalar(
                            out=dst,
                            in0=ps,
                            scalar1=pnorms[:, t : t + 1],
                            scalar2=None,
                            op0=mybir.AluOpType.add,
                        )
                    else:
                        nc.scalar.activation(
                            out=dst,
                            in_=ps,
                            func=mybir.ActivationFunctionType.Relu,
                            bias=pnorms[:, t : t + 1],
                        )

            def fold_rows(dsb, n, t0, i0=0):
                # deep bf16 fold-tree over n packed tiles (3D APs keep 2x)
                cur = dsb.rearrange("p (i m) -> p i m", m=M)[:, i0 : i0 + n, :]
                w = M // 2
                for ft in ftiles:
                    nc.vector.tensor_tensor(
                        out=ft[:, 0:n, :],
                        in0=cur[:, :, 0:w],
                        in1=cur[:, :, w : 2 * w],
                        op=mybir.AluOpType.min,
                    )
                    cur = ft[:, 0:n, :]
                    w //= 2
                nc.vector.tensor_reduce(
                    out=rowmins[:, t0 : t0 + n],
                    in_=cur,
                    axis=mybir.AxisListType.X,
                    op=mybir.AluOpType.min,
                )

            def fold_half(region, out_ap):
                cur = region
                w = M // 4
                for ft in ftiles[1:]:
                    nc.vector.tensor_tensor(
                        out=ft[:, 0, 0:w],
                        in0=cur[:, 0:w],
                        in1=cur[:, w : 2 * w],
                        op=mybir.AluOpType.min,
                    )
                    cur = ft[:, 0, 0:w]
                    w //= 2
                nc.vector.tensor_reduce(
                    out=out_ap,
                    in_=cur,
                    axis=mybir.AxisListType.X,
                    op=mybir.AluOpType.min,
                )

            def col_update(dsb, i0):
                nc.vector.tensor_tensor(
                    out=colacc2_v,
                    in0=dsb.rearrange("p (i m) -> p i m", m=M)[:, i0 : i0 + 2, :],
                    in1=colacc2_v,
                    op=mybir.AluOpType.min,
                )

            if True:
                # t = 0,1 write the two col accumulators directly; fold each
                # singly so the DVE starts as soon as tile 0 is copied
                def emit_half(t, dsb, base, h):
                    ps = psp.tile([128, 2048], F32, name="ps_main", tag="ps_main")
                    for g in range(4):
                        col0 = 2048 * h + 512 * g
                        nc.tensor.matmul(
                            ps[:, 512 * g : 512 * (g + 1)],
                            zp_pk[0:K, 128 * t : 128 * (t + 1)],
                            zg_pk[0:K, col0 : col0 + 512],
                            start=True,
                            stop=True,
                            tile_position=(0, 0),
                        )
                    nc.scalar.activation(
                        out=dsb[:, base + 2048 * h : base + 2048 * (h + 1)],
                        in_=ps,
                        func=mybir.ActivationFunctionType.Relu,
                        bias=pnorms[:, t : t + 1],
                    )

                for t in (0, 1):
                    for h in (0, 1):
                        emit_half(t, colacc2, t * M, h)
                        fold_half(
                            colacc2[:, t * M + h * (M // 2) : t * M + (h + 1) * (M // 2)],
                            rowminsH[:, 2 * t + h : 2 * t + h + 1],
                        )
                # tiles 2-5 as pairs (finer fill granularity while the ACT
                # copy pipeline warms up), then 6 quads; tiles 30-31 run as
                # singles against the pre-merged accumulator so the end-chain
                # after the last RELU is one short col op, not pair+merge
                groups = [(t, 4) for t in range(2, 30, 4)]
                for t0, n in groups:
                    dsb = dsbp.tile([128, 4 * M], BF16, name="dsb", tag="dsb")
                    for i in range(n):
                        emit_spans(t0 + i, dsb, i * M)
                    col_update(dsb, 0)
                    fold_rows(dsb, n, t0)
                    if n == 4:
                        col_update(dsb, 2)
                # merge the paired accumulators while tiles 30-31 stream
                nc.vector.tensor_tensor(
                    out=colacc,
                    in0=colacc2[:, 0:M],
                    in1=colacc2[:, M : 2 * M],
                    op=mybir.AluOpType.min,
                )
                # tiles 30-31 in one buffer: col updates as full (30) +
                # halves (31) so the lo-half transposes can start early,
                # rows as one paired fold after the col chain
                dsb31 = dsbp.tile([128, 4 * M], BF16, name="dsb", tag="dsb")
                emit_spans(30, dsb31, 0)
                nc.vector.tensor_tensor(
                    out=colacc,
                    in0=dsb31[:, 0:M],
                    in1=colacc,
                    op=mybir.AluOpType.min,
                )
                emit_spans(31, dsb31, M)
                H = M // 2
                for hh in range(2):
                    nc.vector.tensor_tensor(
                        out=colacc[:, hh * H : (hh + 1) * H],
                        in0=dsb31[:, M + hh * H : M + (hh + 1) * H],
                        in1=colacc[:, hh * H : (hh + 1) * H],
                        op=mybir.AluOpType.min,
                    )

            # ---- tail: min over partitions of colacc via PE transpose ----
            # 2 groups of 16 [128,128] blocks, each group one half-span, so
            # the reduce is one [128,16,128] op and group 0 overlaps the
            # hi-half col update + tile-31 row fold on the DVE
            colmins = rowcol[:, NT : 2 * NT]
            if True:
                psp2 = psp
                for g in range(2):
                    pst = psp2.tile([128, 2048], BF16, name="ps_tr", tag="ps_main")
                    for k in range(16):
                        b = 16 * g + k
                        nc.tensor.matmul(
                            pst[:, 128 * k : 128 * (k + 1)],
                            colacc[:, 128 * b : 128 * (b + 1)],
                            ident16,
                            is_transpose=True,
                            start=True,
                            stop=True,
                        )
                    if g == 0:
                        fold_rows(dsb31, 2, 30)
                    # ACT (idle at the tail) lifts the transposed blocks to
                    # SBUF so the reduce runs as a 2x bf16 fold + short reduce
                    tsb = persist.tile([128, 2048], BF16, name=f"tsb{g}")
                    nc.scalar.copy(tsb, pst)
                    tsb3 = tsb.rearrange("p (k i) -> p k i", i=128)
                    nc.vector.tensor_tensor(
                        out=tsb3[:, :, 0:64],
                        in0=tsb3[:, :, 0:64],
                        in1=tsb3[:, :, 64:128],
                        op=mybir.AluOpType.min,
                    )
                    nc.vector.tensor_reduce(
                        out=colmins[:, 16 * g : 16 * (g + 1)],
                        in_=tsb3[:, :, 0:64],
                        axis=mybir.AxisListType.X,
                        op=mybir.AluOpType.min,
                    )

                # ---- final sums (minima already clamped: ACT relu-copies) ----
                rowmins_h2 = rowminsH.rearrange("p (t h) -> p t h", h=2)
                nc.vector.tensor_tensor(
                    out=rowmins[:, 0:2],
                    in0=rowmins_h2[:, :, 0],
                    in1=rowmins_h2[:, :, 1],
                    op=mybir.AluOpType.min,
                )
                total = persist.tile([128, 1], F32)
                nc.vector.tensor_reduce(
                    out=total,
                    in_=rowcol,
                    axis=mybir.AxisListType.X,
                    op=mybir.AluOpType.add,
                )
                ps_s = psp2.tile([1, 1], F32, name="ps_s", tag="ps_main")
                ones = nc.const_aps.tensor(1.0, (128, 1))
                nc.tensor.matmul(ps_s, ones, total, start=True, stop=True)
                res_sb = persist.tile([1, 1], F32)
                nc.scalar.mul(res_sb, ps_s, 1.0 / float(N))
                nc.sync.dma_start(out=out_d.ap(), in_=res_sb)
            foldp.release()
            psp.release()

    _split_multi_waits(nc)
    return nc


_NC = None


def _get_nc():
    global _NC
    if _NC is None:
        _NC = build_nc()
    return _NC


def _ensure_ntff_hook():
    """Register the axon NTFF profiling hook if the container's antenv stub
    lacks axon_hooks (trace support; harmless to skip)."""
    import types

    try:
        import antenv
    except ImportError:
        return
    if not hasattr(antenv, "axon_hooks") or not hasattr(
        getattr(antenv, "axon_hooks", None), "get_axon_ntff_profile_hook"
    ):
        mod = types.ModuleType("antenv.axon_hooks")
        mod._h = None
        mod.set_axon_ntff_profile_hook = lambda h: setattr(mod, "_h", h)
        mod.get_axon_ntff_profile_hook = lambda: mod._h
        sys.modules["antenv.axon_hooks"] = mod
        antenv.axon_hooks = mod
    from antenv import axon_hooks

    if axon_hooks.get_axon_ntff_profile_hook() is None:
        try:
            from trn_agent_boot.trn_boot import _ntff_profile_via_ctypes

            hook = _ntff_profile_via_ctypes("/opt/axon/libaxon_pjrt.so")
            if hook is not None:
                axon_hooks.set_axon_ntff_profile_hook(hook)
        except Exception:
            pass


def kernel(pred_points: np.ndarray, gt_points: np.ndarray, _want_trace: bool = False):
    pred = np.ascontiguousarray(np.asarray(pred_points, dtype=np.float32))
    gt = np.ascontiguousarray(np.asarray(gt_points, dtype=np.float32))
    assert pred.shape == (B, N, D) and gt.shape == (B, M, D)

    in_maps = []
    for b in range(B):
        p, g = pred[b], gt[b]
        in_maps.append(
            {
                "predW": np.ascontiguousarray(
                    p.reshape(NT, 128, 3).transpose(1, 0, 2).reshape(128, 3 * NT)
                ),
                "gtW": np.ascontiguousarray(
                    g.reshape(NT, 128, 3).transpose(1, 0, 2).reshape(128, 3 * NT)
                ),
            }
        )

    nc = _get_nc()
    if _want_trace:
        _ensure_ntff_hook()
    res = run_bass_kernel_spmd(nc, in_maps, core_ids=list(range(B)), trace=_want_trace)
    per_core = np.array([r["out"][0, 0] for r in res.results], dtype=np.float64)
    loss = np.float32(per_core.mean())
    if _want_trace:
        return loss, res
    return loss



# revision 55
# speedup vs baseline: 1.0141x; 1.0141x over previous
"""Chamfer loss kernel for Trainium2 (8 NeuronCores, data-parallel over batch).

Problem: B=8, N=M=4096, D=3 fp32 point clouds.
  loss = mean_b mean_n min_m ||p_bn - g_bm||^2  +  mean_b mean_m min_n ||.||^2
  (squared euclidean, clamped at 0, matching pytorch3d norm=2 semantics)

Strategy (one batch element per core):
  - Distance tiles d[n, m] come from K=5 float32r matmuls on the PE:
       k0: 1 * |g|^2_hi   k1: 1 * |g|^2_lo   k2-4: (-2 p_d) * g_d
    The pred-point squared norm |p_n|^2 is constant per OUTPUT ROW
    (= PSUM partition), so it rides the ACT relu-copy's per-partition
    bias port in exact fp32 instead of costing two more f32r strip rows
    (and the whole pred-side norm hi/lo+transpose prep chain). f32r keeps
    ~12 mantissa bits; the gt norms are hi/lo split so d's error is only
    the coordinate-rounding perturbation (~1e-5 relative on the loss).
  - The K=5 operand strips are replicated at the 4 PE row-group bases
    (partition 32g) via 6 SBUF-SBUF DMAs on the SP/GPSIMD queues; each
    half-span's 4 matmuls then come from 4 different row groups, so the
    PE reorder window hides LDWEIGHTS (PE busy ~131us unpacked -> ~70us).
    Tiles t<6 run unpacked from row group 0 so nothing waits on the
    replication; doorbells live only on the SP/GPSIMD sequencers because
    a doorbell's dependency wait stalls its whole in-order sequencer.
  - ACT relu-copies each PSUM half-span to SBUF bf16 (clamp + |p|^2 bias
    fused; ~3.7us/tile, hidden under the DVE).
  - Row minima: deep bf16 tensor_tensor fold-tree (2x mode) over groups
    of 2-4 tiles down to width 64, then one 1x tensor_reduce
    (~2.2-2.3us/tile). Tiles 0/1 fold per half-span so the DVE starts on
    the very first RELU.
  - Column minima: [128, 2, M] tensor_tensor min into two interleaved
    accumulators (~2.2us/tile) written directly by tiles 0/1's RELUs (no
    init). They merge while tiles 30-31 stream; those two tiles update
    the merged accumulator as singles (tile 31 in halves) so the
    end-chain after the last RELU is short. Partition-axis min: 2 groups
    of 16 PE transposes; ACT lifts each group to SBUF so the reduce runs
    as a 2x bf16 fold + short reduce. Row+col sums fuse into one reduce.
  - The DVE is the bottleneck engine (~151us busy, ~99% dense from the
    first fold to the end); ACT ~122us; PE ~70us. Engine-assignment notes
    for this toolchain: tensor_tensor/tensor_reduce(free-axis) are
    DVE-only (Pool rejects them in walrus codegen), native
    TENSOR_TENSOR_REDUCE min/min fails ISA encoding, DMA CCE accum
    supports no min, InstPool max measures 1.6 cyc/elem (slower than the
    fold tree), matmul bf16 PSUM output is TRN3-only, and ACT's
    accumulator is sum-only - so the bf16 fold-tree/col-acc floor
    (~4.4us/tile, at the DVE write-port limit) is the steady-state wall.
  - Per-core scalar output (cham_x_b + cham_y_b); the host averages the 8
    per-core scalars (the data-parallel gather).

All arithmetic happens on-chip; the host only reshapes/transposes inputs
(layout) and averages the per-core partial losses (unshard).
"""

import os
import sys

import numpy as np

sys.path.insert(0, "/opt/trn_rl_repo")

import bass_rust
import concourse.bass as bass
import concourse.mybir as mybir
from concourse.bass_utils import run_bass_kernel_spmd
from concourse.masks import make_identity
from concourse.tile import TileContext

B, N, M, D = 8, 4096, 4096, 3
NT = N // 128  # 32 n-tiles
K = 5
F32 = mybir.dt.float32
F32R = mybir.dt.float32r
BF16 = mybir.dt.bfloat16
BIG = 3.0e38

# ---------------------------------------------------------------------------
# walrus in this container rejects >1 sync-wait per instruction; spill the
# extras onto engine-matched NoOps placed immediately before the instruction.
_nop_counter = [0]


def _split_multi_waits(nc):
    for func in nc.m.functions:
        for bb in func.blocks:
            out = []
            dirty = False
            for inst in bb.instructions:
                si = inst.sync_info
                if si is not None and len(si.on_wait) > 1:
                    waits = list(si.on_wait)
                    for w in waits[:-1]:
                        _nop_counter[0] += 1
                        nop = mybir.InstNoOp(
                            name=f"I-waitsplit-{_nop_counter[0]}", ins=[], outs=[]
                        )
                        nop.engine = inst.engine
                        nop.sync_info = bass_rust.SyncInfo(on_wait=[w], on_update=[])
                        out.append(nop)
                    inst.sync_info = bass_rust.SyncInfo(
                        on_wait=[waits[-1]], on_update=list(si.on_update)
                    )
                    dirty = True
                out.append(inst)
            if dirty:
                bb.instructions = out
    return nc


# ---------------------------------------------------------------------------


_PREP_WR = {}


def _build_prep_side(nc, tc, pool, zpk, w_dram, ident, scale, c_row, q, psp, strip_norms):
    """Build one side's coord rows (and optionally norm rows) of zpk
    [96+K, 4096] f32r.

    Everything derives from the wide input layout [128, 96] (point 128t+p at
    partition p, cols 3t..3t+2): rounded coords via PE-transpose flattened
    into n-order by ONE 3-row reshape DMA. For the gt side (strip_norms),
    the squared norms are hi/lo-split into f32r strip rows 0-1; for the pred
    side the wide fp32 norms tile is returned instead and applied later as
    the per-partition bias of the ACT relu-copy (exact fp32, no strip rows).
    Row groups 1-3 are filled by the replication hops in build_nc.
    """
    nm = w_dram.name
    wr = _PREP_WR[nm]  # rounded+scaled wide input, loaded up front

    # --- coords: transpose wr -> [96, 128], one 3-row reshape DMA ---
    tw_ps = psp.tile([128, 128], F32, name=f"twps_{nm}", tag="ps_main")
    nc.tensor.matmul(
        tw_ps[0 : 3 * NT, :],
        wr.bitcast(F32),
        ident,
        is_transpose=True,
        start=True,
        stop=True,
    )
    tw = pool.tile([3 * NT, 128], F32R, name=f"tw_{nm}")
    nc.vector.tensor_copy(tw, tw_ps[0 : 3 * NT, :])
    tw_d = tw.rearrange("(t d) p -> d t p", d=3)
    for d in range(3):
        q.dma_start(out=zpk[c_row + d : c_row + d + 1, :], in_=tw_d[d])

    # --- norms of the rounded points ---
    wsq = pool.tile([128, 3 * NT], F32, name=f"wsq_{nm}")
    nc.vector.tensor_mul(wsq, wr, wr)
    norms = pool.tile([128, NT], F32, name=f"norms_{nm}")
    nc.vector.tensor_reduce(
        out=norms,
        in_=wsq.rearrange("p (t d) -> p t d", d=3),
        axis=mybir.AxisListType.X,
        op=mybir.AluOpType.add,
    )
    if scale != 1.0:
        # norms of scale*p -> divide by scale^2 (exact for powers of 2)
        nc.vector.tensor_scalar(
            out=norms,
            in0=norms,
            scalar1=1.0 / (scale * scale),
            scalar2=None,
            op0=mybir.AluOpType.mult,
        )
    if not strip_norms:
        return norms
    nh = pool.tile([128, NT], F32R, name=f"nh_{nm}")
    nc.vector.tensor_copy(nh, norms)
    nl_f = pool.tile([128, NT], F32, name=f"nlf_{nm}")
    nc.vector.tensor_sub(nl_f, norms, nh.bitcast(F32))
    nl = pool.tile([128, NT], F32R, name=f"nl_{nm}")
    nc.vector.tensor_copy(nl, nl_f)
    tn2 = pool.tile([2 * NT, 128], F32R, name=f"tn2_{nm}")
    for i, src in enumerate((nh, nl)):
        tn_ps = psp.tile([128, 128], F32, name=f"tnps_{nm}_{i}", tag="ps_main")
        nc.tensor.matmul(
            tn_ps[0:NT, :],
            src.bitcast(F32),
            ident,
            is_transpose=True,
            start=True,
            stop=True,
        )
        nc.vector.tensor_copy(tn2[NT * i : NT * (i + 1), :], tn_ps[0:NT, :])
    q.dma_start(out=zpk[0:2, :], in_=tn2)
    return None


def build_nc():
    nc = bass.Bass("TRN2")
    predW = nc.dram_tensor("predW", [128, 3 * NT], F32, kind="ExternalInput")
    gtW = nc.dram_tensor("gtW", [128, 3 * NT], F32, kind="ExternalInput")
    out_d = nc.dram_tensor("out", [1, 1], F32, kind="ExternalOutput")

    with TileContext(nc) as tc:
        with (
            tc.tile_pool(name="persist", bufs=1) as persist,
            tc.tile_pool(name="dsb", bufs=2) as dsbp,
        ):
            # load + round both wide inputs first — everything derives from
            # them, so they must not queue behind prep DMAs
            for w_dram, scale, q in ((predW, -2.0, nc.sync), (gtW, 1.0, nc.scalar)):
                w_in = persist.tile([128, 3 * NT], F32, name=f"w_{w_dram.name}")
                q.dma_start(out=w_in, in_=w_dram.ap())
                wr_t = persist.tile([128, 3 * NT], F32R, name=f"wr_{w_dram.name}")
                nc.vector.tensor_scalar(
                    out=wr_t,
                    in0=w_in,
                    scalar1=scale,
                    scalar2=None,
                    op0=mybir.AluOpType.mult,
                )
                _PREP_WR[w_dram.name] = wr_t
            # identity zero-fill on DVE so GPSIMD's single queue only does
            # the diagonal writes (keeps the prep critical path short)
            ident = persist.tile([128, 128], F32)
            nc.vector.memset(ident, 0.0)
            make_identity(nc, ident, nomemset=True)
            ident16 = persist.tile([128, 128], BF16)
            nc.vector.memset(ident16, 0.0)
            make_identity(nc, ident16, nomemset=True)

            # K=5 operand strips, replicated at the 4 PE row-group bases so
            # consecutive matmuls hide their LDWEIGHTS across row groups.
            zp_pk = persist.tile([96 + K, N], F32R)
            zg_pk = persist.tile([96 + K, M], F32R)
            # ones rows (Memset can't target f32r; compute ops can't start at
            # unaligned partitions): rounded ones strip -> broadcast DMA.
            # ones rows, built wide (128 lanes, ~100ns) then DMA'd out
            ones_w = persist.tile([128, 64], F32)
            nc.vector.memset(ones_w, 1.0)
            ones_wr = persist.tile([128, 64], F32R)
            nc.vector.tensor_copy(ones_wr, ones_w)
            nc.sync.dma_start(out=zp_pk[0:2, :], in_=ones_wr)
            # zp rows: [1, 1, -2p0, -2p1, -2p2]   (pnorm applied as ACT bias)
            # zg rows: [nGh, nGl, g0, g1, g2]
            psp = tc.alloc_tile_pool(name="psum_all", bufs=2, space="PSUM")
            _build_prep_side(
                nc, tc, persist, zg_pk, gtW, ident, 1.0, 2,
                nc.scalar, psp, strip_norms=True,
            )
            pnorms = _build_prep_side(
                nc, tc, persist, zp_pk, predW, ident, -2.0, 2,
                nc.sync, psp, strip_norms=False,
            )
            # replicate both K-strips at PE row-group bases 32/64/96: the 4
            # matmuls of each half-span then come from 4 different row groups,
            # so the PE reorder window hides each group's LDWEIGHTS under the
            # others' streams (PE busy ~131us unpacked -> ~59us packed)
            # doorbells only on SP/GPSIMD queues: a doorbell's dependency wait
            # blocks its whole (in-order) sequencer, and Scalar must keep
            # issuing the main-loop ACT copies
            repl_q = [nc.sync, nc.gpsimd, nc.gpsimd, nc.sync, nc.sync, nc.gpsimd]
            for g in range(1, 4):
                repl_q[2 * (g - 1)].dma_start(
                    out=zp_pk[32 * g : 32 * g + K, :], in_=zp_pk[0:K, :]
                )
                repl_q[2 * (g - 1) + 1].dma_start(
                    out=zg_pk[32 * g : 32 * g + K, :], in_=zg_pk[0:K, :]
                )

            rowcol = persist.tile([128, 2 * NT], F32)
            rowmins = rowcol[:, 0:NT]
            # half-tile row-min slots for tiles 0/1 (folded per half-span so
            # the DVE starts as soon as the very first RELU lands)
            rowminsH = persist.tile([128, 4], F32)
            # two independent column accumulators (updated as one [128,2,M]
            # tensor_tensor, halving the op count) merged once at the end
            colacc2 = persist.tile([128, 2 * M], BF16)
            colacc = persist.tile([128, M], BF16)
            colacc2_v = colacc2.rearrange("p (i m) -> p i m", m=M)
            # fold-tree intermediates: written+read only by the DVE (in-order)
            # so single buffers suffice; sized for the max group of 4 tiles
            foldp = tc.alloc_tile_pool(name="foldp", bufs=1)
            ftiles = [
                foldp.tile([128, 4, w], BF16, name=f"f{w}")
                for w in (2048, 1024, 512, 256, 128, 64)
            ]

            # ---- main loop: 32 n-tiles x 2 half-spans of [128, 2048];
            #      each span = 4 row-group-packed concurrent matmuls ----
            # Pipeline per n-tile: PE (4 packed MMs per half-span) -> ACT
            # relu-copies PSUM->SBUF bf16 (clamp fused, so no tail clamps)
            # -> DVE row path: deep bf16 fold-tree (2x mode) over groups of
            # 4 tiles down to width 64 before the (1x-rate) tensor_reduce;
            # col path: [128,2,M] tensor_tensor min into the paired
            # accumulators. Steady-state DVE ~4.4us/tile vs ~4.7 baseline.
            def emit_spans(t, dsb, base, dve_h1=False):
                # early tiles run unpacked from row group 0 so they don't
                # wait on the replication DMAs; later tiles rotate across
                # the 4 row groups (LDWEIGHTS hidden, PE ~60us busy)
                packed = t >= 6
                for h in range(2):
                    ps = psp.tile([128, 2048], F32, name="ps_main", tag="ps_main")
                    for g in range(4):
                        col0 = 2048 * h + 512 * g
                        b = 32 * g if packed else 0
                        nc.tensor.matmul(
                            ps[:, 512 * g : 512 * (g + 1)],
                            zp_pk[b : b + K, 128 * t : 128 * (t + 1)],
                            zg_pk[b : b + K, col0 : col0 + 512],
                            start=True,
                            stop=True,
                            tile_position=(b, 0),
                        )
                    dst = dsb[:, base + 2048 * h : base + 2048 * (h + 1)]
                    # the pred-point squared norm is constant per output row
                    # (= partition), so it rides the ACT copy's bias port in
                    # exact fp32 instead of costing two f32r strip rows
                    if dve_h1 and h == 1:
                        # warmup assist: the DVE converts this half during a
                        # window where it would otherwise idle waiting on ACT
                        # deliveries, pulling the whole schedule forward. No
                        # relu here - the final rowcol clamp covers it.
                        nc.vector.tensor_scalar(
                            out=dst,
                            in0=ps,
                            scalar1=pnorms[:, t : t + 1],
                            scalar2=None,
                            op0=mybir.AluOpType.add,
                        )
                    else:
                        nc.scalar.activation(
                            out=dst,
                            in_=ps,
                            func=mybir.ActivationFunctionType.Relu,
                            bias=pnorms[:, t : t + 1],
                        )

            def fold_rows(dsb, n, t0, i0=0):
                # deep bf16 fold-tree over n packed tiles (3D APs keep 2x)
                cur = dsb.rearrange("p (i m) -> p i m", m=M)[:, i0 : i0 + n, :]
                w = M // 2
                for ft in ftiles:
                    nc.vector.tensor_tensor(
                        out=ft[:, 0:n, :],
                        in0=cur[:, :, 0:w],
                        in1=cur[:, :, w : 2 * w],
                        op=mybir.AluOpType.min,
                    )
                    cur = ft[:, 0:n, :]
                    w //= 2
                nc.vector.tensor_reduce(
                    out=rowmins[:, t0 : t0 + n],
                    in_=cur,
                    axis=mybir.AxisListType.X,
                    op=mybir.AluOpType.min,
                )

            def fold_half(region, out_ap):
                cur = region
                w = M // 4
                for ft in ftiles[1:]:
                    nc.vector.tensor_tensor(
                        out=ft[:, 0, 0:w],
                        in0=cur[:, 0:w],
                        in1=cur[:, w : 2 * w],
                        op=mybir.AluOpType.min,
                    )
                    cur = ft[:, 0, 0:w]
                    w //= 2
                nc.vector.tensor_reduce(
                    out=out_ap,
                    in_=cur,
                    axis=mybir.AxisListType.X,
                    op=mybir.AluOpType.min,
                )

            def col_update(dsb, i0):
                nc.vector.tensor_tensor(
                    out=colacc2_v,
                    in0=dsb.rearrange("p (i m) -> p i m", m=M)[:, i0 : i0 + 2, :],
                    in1=colacc2_v,
                    op=mybir.AluOpType.min,
                )

            if True:
                # t = 0,1 write the two col accumulators directly; fold each
                # singly so the DVE starts as soon as tile 0 is copied
                def emit_half(t, dsb, base, h):
                    ps = psp.tile([128, 2048], F32, name="ps_main", tag="ps_main")
                    for g in range(4):
                        col0 = 2048 * h + 512 * g
                        nc.tensor.matmul(
                            ps[:, 512 * g : 512 * (g + 1)],
                            zp_pk[0:K, 128 * t : 128 * (t + 1)],
                            zg_pk[0:K, col0 : col0 + 512],
                            start=True,
                            stop=True,
                            tile_position=(0, 0),
                        )
                    nc.scalar.activation(
                        out=dsb[:, base + 2048 * h : base + 2048 * (h + 1)],
                        in_=ps,
                        func=mybir.ActivationFunctionType.Relu,
                        bias=pnorms[:, t : t + 1],
                    )

                for t in (0, 1):
                    for h in (0, 1):
                        emit_half(t, colacc2, t * M, h)
                        fold_half(
                            colacc2[:, t * M + h * (M // 2) : t * M + (h + 1) * (M // 2)],
                            rowminsH[:, 2 * t + h : 2 * t + h + 1],
                        )
                # tiles 2-5 as pairs (finer fill granularity while the ACT
                # copy pipeline warms up), then 6 quads; tiles 30-31 run as
                # singles against the pre-merged accumulator so the end-chain
                # after the last RELU is one short col op, not pair+merge
                groups = [(2, 2), (4, 2), (6, 2), (8, 2)] + [(t, 4) for t in range(10, 30, 4)]
                for t0, n in groups:
                    dsb = dsbp.tile([128, 4 * M], BF16, name="dsb", tag="dsb")
                    for i in range(n):
                        emit_spans(t0 + i, dsb, i * M)
                    col_update(dsb, 0)
                    fold_rows(dsb, n, t0)
                    if n == 4:
                        col_update(dsb, 2)
                # merge the paired accumulators while tiles 30-31 stream
                nc.vector.tensor_tensor(
                    out=colacc,
                    in0=colacc2[:, 0:M],
                    in1=colacc2[:, M : 2 * M],
                    op=mybir.AluOpType.min,
                )
                # tiles 30-31 in one buffer: col updates as full (30) +
                # halves (31) so the lo-half transposes can start early,
                # rows as one paired fold after the col chain
                dsb31 = dsbp.tile([128, 4 * M], BF16, name="dsb", tag="dsb")
                emit_spans(30, dsb31, 0)
                nc.vector.tensor_tensor(
                    out=colacc,
                    in0=dsb31[:, 0:M],
                    in1=colacc,
                    op=mybir.AluOpType.min,
                )
                emit_spans(31, dsb31, M)
                H = M // 2
                for hh in range(2):
                    nc.vector.tensor_tensor(
                        out=colacc[:, hh * H : (hh + 1) * H],
                        in0=dsb31[:, M + hh * H : M + (hh + 1) * H],
                        in1=colacc[:, hh * H : (hh + 1) * H],
                        op=mybir.AluOpType.min,
                    )

            # ---- tail: min over partitions of colacc via PE transpose ----
            # 2 groups of 16 [128,128] blocks, each group one half-span, so
            # the reduce is one [128,16,128] op and group 0 overlaps the
            # hi-half col update + tile-31 row fold on the DVE
            colmins = rowcol[:, NT : 2 * NT]
            if True:
                psp2 = psp
                for g in range(2):
                    pst = psp2.tile([128, 2048], BF16, name="ps_tr", tag="ps_main")
                    for k in range(16):
                        b = 16 * g + k
                        nc.tensor.matmul(
                            pst[:, 128 * k : 128 * (k + 1)],
                            colacc[:, 128 * b : 128 * (b + 1)],
                            ident16,
                            is_transpose=True,
                            start=True,
                            stop=True,
                        )
                    if g == 0:
                        fold_rows(dsb31, 2, 30)
                    # ACT (idle at the tail) lifts the transposed blocks to
                    # SBUF so the reduce runs as a 2x bf16 fold + short reduce
                    tsb = persist.tile([128, 2048], BF16, name=f"tsb{g}")
                    nc.scalar.copy(tsb, pst)
                    tsb3 = tsb.rearrange("p (k i) -> p k i", i=128)
                    nc.vector.tensor_tensor(
                        out=tsb3[:, :, 0:64],
                        in0=tsb3[:, :, 0:64],
                        in1=tsb3[:, :, 64:128],
                        op=mybir.AluOpType.min,
                    )
                    nc.vector.tensor_reduce(
                        out=colmins[:, 16 * g : 16 * (g + 1)],
                        in_=tsb3[:, :, 0:64],
                        axis=mybir.AxisListType.X,
                        op=mybir.AluOpType.min,
                    )

                # ---- final sums (minima already clamped: ACT relu-copies) ----
                rowmins_h2 = rowminsH.rearrange("p (t h) -> p t h", h=2)
                nc.vector.tensor_tensor(
                    out=rowmins[:, 0:2],
                    in0=rowmins_h2[:, :, 0],
                    in1=rowmins_h2[:, :, 1],
                    op=mybir.AluOpType.min,
                )
                total = persist.tile([128, 1], F32)
                nc.vector.tensor_reduce(
                    out=total,
                    in_=rowcol,
                    axis=mybir.AxisListType.X,
                    op=mybir.AluOpType.add,
                )
                ps_s = psp2.tile([1, 1], F32, name="ps_s", tag="ps_main")
                ones = nc.const_aps.tensor(1.0, (128, 1))
                nc.tensor.matmul(ps_s, ones, total, start=True, stop=True)
                res_sb = persist.tile([1, 1], F32)
                nc.scalar.mul(res_sb, ps_s, 1.0 / float(N))
                nc.sync.dma_start(out=out_d.ap(), in_=res_sb)
            foldp.release()
            psp.release()

    _split_multi_waits(nc)
    return nc


_NC = None


def _get_nc():
    global _NC
    if _NC is None:
        _NC = build_nc()
    return _NC


def _ensure_ntff_hook():
    """Register the axon NTFF profiling hook if the container's antenv stub
    lacks axon_hooks (trace support; harmless to skip)."""
    import types

    try:
        import antenv
    except ImportError:
        return
    if not hasattr(antenv, "axon_hooks") or not hasattr(
        getattr(antenv, "axon_hooks", None), "get_axon_ntff_profile_hook"
    ):
        mod = types.ModuleType("antenv.axon_hooks")
        mod._h = None
        mod.set_axon_ntff_profile_hook = lambda h: setattr(mod, "_h", h)
        mod.get_axon_ntff_profile_hook = lambda: mod._h
        sys.modules["antenv.axon_hooks"] = mod
        antenv.axon_hooks = mod
    from antenv import axon_hooks

    if axon_hooks.get_axon_ntff_profile_hook() is None:
        try:
            from trn_agent_boot.trn_boot import _ntff_profile_via_ctypes

            hook = _ntff_profile_via_ctypes("/opt/axon/libaxon_pjrt.so")
            if hook is not None:
                axon_hooks.set_axon_ntff_profile_hook(hook)
        except Exception:
            pass


def kernel(pred_points: np.ndarray, gt_points: np.ndarray, _want_trace: bool = False):
    pred = np.ascontiguousarray(np.asarray(pred_points, dtype=np.float32))
    gt = np.ascontiguousarray(np.asarray(gt_points, dtype=np.float32))
    assert pred.shape == (B, N, D) and gt.shape == (B, M, D)

    in_maps = []
    for b in range(B):
        p, g = pred[b], gt[b]
        in_maps.append(
            {
                "predW": np.ascontiguousarray(
                    p.reshape(NT, 128, 3).transpose(1, 0, 2).reshape(128, 3 * NT)
                ),
                "gtW": np.ascontiguousarray(
                    g.reshape(NT, 128, 3).transpose(1, 0, 2).reshape(128, 3 * NT)
                ),
            }
        )

    nc = _get_nc()
    if _want_trace:
        _ensure_ntff_hook()
    res = run_bass_kernel_spmd(nc, in_maps, core_ids=list(range(B)), trace=_want_trace)
    per_core = np.array([r["out"][0, 0] for r in res.results], dtype=np.float64)
    loss = np.float32(per_core.mean())
    if _want_trace:
        return loss, res
    return loss

